# revision 58
# baseline (speedup 1.0000x reference)
"""Trainium2 Bass kernel for nn_Attention_28802050687173.

Channel-attention block: 1x1 conv (c->4c), depthwise 3x3, gating multiply,
L2-normalized channel gram + softmax, attn @ v, 1x1 conv out.

Sharding: 8 cores = (sample, H-half).  Each core processes 128 rows x 256 cols
of one sample (n_loc = 32768 pixels).  The depthwise conv is folded into the
input projection: dw = sum_j (w_dw[:,j] * W_in1) @ x_shift_j, so the whole
front end is 7 matmuls per tile over a zero-padded, duplicated+shifted copy of
x built host-side.  The channel gram S = [v;q][v;q]^T is accumulated on-chip
(PE transposes + bf16 matmuls).  To keep the gram AllReduce off the critical
path, each core redundantly computes the gram contribution of its PARTNER's
last 8 tiles (2 extra input windows): the collectives then only cover tiles
0..55 and complete under the tail compute.  Softmax and the fused
(w_out @ attn) @ v output projection follow, stored in bf16.
"""
import numpy as np

import concourse.bass as bass
import concourse.mybir as mybir
import concourse.tile as tile
from concourse import bacc
from concourse.bass_utils import run_bass_kernel_spmd
from concourse.masks import make_identity

F32 = mybir.dt.float32
F32R = mybir.dt.float32r
BF16 = mybir.dt.bfloat16


def _install_ntff_hook():
    """The container's antenv stub lacks axon_hooks, so bass_utils'
    trace=True path can't find the NTFF profile hook the axon .so
    provides.  Recreate the hook (same ctypes ABI trn_agent_boot uses)
    and inject an antenv.axon_hooks module exposing it."""
    import sys
    import contextlib
    import ctypes
    if "antenv.axon_hooks" in sys.modules:
        return
    so_path = "/opt/axon/libaxon_pjrt.so"
    try:
        lib = ctypes.CDLL(so_path)
    except OSError:
        return
    if not hasattr(lib, "axon_start_nrt_profile"):
        return
    lib.axon_start_nrt_profile.argtypes = [
        ctypes.POINTER(ctypes.c_int64), ctypes.c_size_t]
    lib.axon_start_nrt_profile.restype = ctypes.c_int64
    lib.axon_stop_nrt_profile.argtypes = [ctypes.c_char_p]
    lib.axon_stop_nrt_profile.restype = ctypes.c_int64

    @contextlib.contextmanager
    def _hook(output_dir, device_ids):
        import jax
        jax.devices()
        if device_ids:
            ids = (ctypes.c_int64 * len(device_ids))(*device_ids)
            rc = lib.axon_start_nrt_profile(ids, len(device_ids))
        else:
            rc = lib.axon_start_nrt_profile(None, 0)
        if rc != 0:
            raise RuntimeError(f"axon_start_nrt_profile rc={rc}")
        try:
            yield
        finally:
            n = lib.axon_stop_nrt_profile(str(output_dir).encode())
            if n < 0:
                raise RuntimeError(f"axon_stop_nrt_profile rc={n}")

    import types
    mod = types.ModuleType("antenv.axon_hooks")
    mod._hook = _hook
    mod.get_axon_ntff_profile_hook = lambda: mod._hook
    mod.set_axon_ntff_profile_hook = lambda h: setattr(mod, "_hook", h)
    sys.modules["antenv.axon_hooks"] = mod
    try:
        import antenv
        antenv.axon_hooks = mod
    except ImportError:
        pass

B, C, H, W = 4, 64, 256, 256
RS = 258                     # zero-padded row stride
HROWS = 130                  # 128 output rows + 1 halo row each side
XLEN = HROWS * RS            # 33540 elements per channel per core
TROWS = 18                   # partner-tail block: 16 output rows + halo
XT = TROWS * RS              # 4644
XLT = XLEN + XT
SHIFT = 259                  # dup-half shift: tap (ky,kx) -> (ky+1,kx+1)
N = 128 * 256                # 32768 outputs per core
NT = 512                     # matmul/psum tile (2 output rows)
WINR = 8                     # output rows per DMA window
WIN = (WINR + 2) * RS        # 2580 elements per window
NWIN = 128 // WINR           # 16 windows
NWIN_T = 2                   # partner-tail windows
SUBT = WINR // 2             # 4 sub-tiles per window
NTILES = N // NT             # 64 own tiles
TTILES = NTILES + NWIN_T * SUBT   # 72 incl. partner tail
SPLIT = 32                   # tiles [0, SPLIT) -> S_a (AllReduce #1)
SPLIT2 = 56                  # tiles [SPLIT, SPLIT2) -> S_b (AllReduce #2)
PAIR_TAPS = [(0, 0), (0, 1), (1, 0)]     # (ky,kx); partner = (ky+1,kx+1)
RG = [[0, 1], [2, 3], [4, 5], [6, 7]]    # AllReduce pairs (same sample)

_CACHE = {}


def _rhs3(xd_t, parts, j, ky, kx, p0=0):
    """[parts, 2, 256] view: output sub-tile j, tap (ky, kx)."""
    v = xd_t[p0:p0 + parts, :].rearrange("p (r c) -> p r c", r=WINR + 2, c=RS)
    return v[:, 2 * j + ky: 2 * j + ky + 2, kx: kx + 256]


def build_nc():
    nc = bacc.Bacc("TRN2", target_bir_lowering=False, debug=False, num_devices=8)

    xd_d = nc.dram_tensor("xd", [128, XLT], BF16, kind="ExternalInput")
    xd3_d = nc.dram_tensor("xd3", [128, XLT], BF16, kind="ExternalInput")
    wp_d = nc.dram_tensor("wp", [128, 3 * 128], BF16, kind="ExternalInput")
    ws_d = nc.dram_tensor("ws", [128, 2 * 128], BF16, kind="ExternalInput")
    w2_d = nc.dram_tensor("w2", [128, 128], BF16, kind="ExternalInput")
    wo_d = nc.dram_tensor("wo", [64, 64], F32, kind="ExternalInput")
    tp_d = nc.dram_tensor("tp", [1, 1], F32, kind="ExternalInput")
    out_d = nc.dram_tensor("out", [64, N], BF16, kind="ExternalOutput")

    with tile.TileContext(nc) as tc:
        from contextlib import ExitStack
        with ExitStack() as outer:
            pool_w = outer.enter_context(tc.tile_pool(name="wts", bufs=1))
            pool_s = outer.enter_context(tc.tile_pool(name="sbuf_s", bufs=1))
            pool_ps_S = outer.enter_context(
                tc.tile_pool(name="ps_S", bufs=1, space="PSUM"))
            pool_dram = outer.enter_context(
                tc.tile_pool(name="dram", bufs=1, space="DRAM"))

            # persistent tiles
            wp_sb = pool_w.tile([128, 3 * 128], BF16)
            ws_sb = pool_w.tile([128, 2 * 128], BF16)
            w2_sb = pool_w.tile([128, 128], BF16)
            wo_sb = pool_w.tile([64, 64], F32)
            tp_sb = pool_w.tile([1, 1], F32)
            id_bf = pool_w.tile([128, 128], BF16)
            s_t = pool_s.tile([128, N], BF16)
            S_all = pool_ps_S.tile([128, 384], F32)
            S_ps = S_all[:, 0:128]
            S_ps_b = S_all[:, 128:256]
            S_ps_c = S_all[:, 256:384]
            cc_in = pool_dram.tile([66, 64], BF16)
            cc_out = pool_dram.tile([66, 64], BF16)
            cc_in_b = pool_dram.tile([66, 64], BF16)
            cc_out_b = pool_dram.tile([66, 64], BF16)
            dmy_in = pool_dram.tile([1, 16], F32)
            dmy_out = pool_dram.tile([1, 16], F32)
            warm_d = pool_dram.tile([1, 16], F32)

            # tiny dummy AllReduce: pays the one-time mesh-algo init on the
            # CC core and absorbs inter-core launch skew while pass 1 runs.
            # Its payload rides HWDGE so the trigger fires within ~5us; a
            # separate throwaway SWDGE DMA pays the ~30us software-DGE
            # cold-start in the background before the real payloads need it.
            dmy_sb = pool_w.tile([1, 16], F32)
            nc.gpsimd.memset(dmy_sb[:], 1.0)
            nc.sync.dma_start(dmy_in[:], dmy_sb[:])
            nc.gpsimd.collective_compute(
                "AllReduce", mybir.AluOpType.add, replica_groups=RG,
                ins=[dmy_in.opt()], outs=[dmy_out.opt()])
            nc.gpsimd.dma_start(warm_d[:], dmy_sb[:])
            # weights ride the scalar queue so the sync queue starts window
            # 0 immediately; wo/tp (needed late) load inside the loop
            nc.scalar.dma_start(wp_sb[:], wp_d[:])
            nc.scalar.dma_start(w2_sb[:], w2_d[:])
            nc.scalar.dma_start(ws_sb[:], ws_d[:])
            make_identity(nc, id_bf[:])
            # preload ACT table sets (exp, abs_rsqrt) so the softmax phase
            # does not pay the ~2.7us-per-set load inside the collective gap
            scr_a = pool_w.tile([1, 1], F32)
            scr_b = pool_w.tile([1, 1], F32)
            nc.scalar.activation(scr_a[:], dmy_sb[0:1, 0:1],
                                 mybir.ActivationFunctionType.Exp)
            nc.scalar.activation(scr_b[:], scr_a[:],
                                 mybir.ActivationFunctionType.Abs_reciprocal_sqrt)
            ones_sb = pool_w.tile([1, 64], F32)
            nc.gpsimd.memset(ones_sb[:], 1.0)
            ones_col = pool_w.tile([128, 1], F32)
            nc.gpsimd.memset(ones_col[:], 1.0)
            tb_sb = pool_w.tile([64, 1], F32)   # temp broadcast column
            # constant f32 diag mask (expanded from bf16 identity)
            diag_msk = pool_w.tile([128, 128], F32)
            nc.scalar.copy(diag_msk[:], id_bf[:])
            # pass-2 weights buffer, zero-padded to K=128
            a2t_bf = pool_w.tile([128, 64], BF16)
            nc.gpsimd.memset(a2t_bf[:], 0.0)

            # ---------------- pass 1: conv front-end + gram ----------------
            # bf16 collective payloads: halves the fabric transfer time of
            # the AllReduces; the gram entries only feed softmax logits so
            # the 0.4% rounding is well inside the error budget
            Sa_sb = pool_w.tile([128, 128], BF16)
            dtmp = pool_w.tile([128, 128], F32)
            diag_a = pool_w.tile([128, 1], BF16)
            diag_b = pool_w.tile([128, 1], BF16)
            diag_c = pool_w.tile([128, 1], F32)
            Sb_sb = pool_w.tile([64, 64], BF16)
            gvq_a = pool_w.tile([64, 64], BF16)  # AR#1 result readback
            sv_a = pool_w.tile([1, 64], BF16)
            sq_a = pool_w.tile([1, 64], BF16)
            # warm the PE HAM before pass 1: a dense burst of dummy
            # matmuls with (almost) no dependencies that runs during the
            # initial DMA waits
            with tc.tile_pool(name="ps_w0", bufs=1, space="PSUM") as pw0:
                warm0 = pw0.tile([128, 128], F32)
                for _ in range(12):
                    nc.tensor.matmul(warm0[:], wp_sb[:, 0:128],
                                     wp_sb[:, 0:128], start=True, stop=True)

            with ExitStack() as p1:
                pool_xd = p1.enter_context(tc.tile_pool(name="xd", bufs=8))
                pool_tb = p1.enter_context(
                    tc.tile_pool(name="ps_tb", bufs=1, space="PSUM"))
                pool_dw = p1.enter_context(
                    tc.tile_pool(name="ps_dw", bufs=3, space="PSUM"))
                pool_x2 = p1.enter_context(
                    tc.tile_pool(name="ps_x2", bufs=1, space="PSUM"))
                pool_tr = p1.enter_context(
                    tc.tile_pool(name="ps_tr", bufs=2, space="PSUM"))
                pool_x2sb = p1.enter_context(tc.tile_pool(name="x2sb", bufs=4))
                pool_st = p1.enter_context(tc.tile_pool(name="stsb", bufs=6))
                pool_sc = p1.enter_context(tc.tile_pool(name="scsb", bufs=3))

                sT_tiles = {}
                sc_tiles = {}

                def src_of(t):
                    if t < NTILES:
                        return s_t[:, NT * t: NT * (t + 1)]
                    return sc_tiles[t][:]

                def emit_transpose(t):
                    src = src_of(t)
                    tr_ps = pool_tr.tile([128, NT], BF16)
                    for q in range(4):
                        nc.tensor.transpose(
                            tr_ps[:, 128 * q: 128 * (q + 1)],
                            src[:, 128 * q: 128 * (q + 1)],
                            id_bf[:])
                    sT_sb = pool_st.tile([128, NT], BF16)
                    nc.vector.tensor_copy(sT_sb[:], tr_ps[:])
                    sT_tiles[t] = sT_sb
                    if t >= NTILES:
                        sc_tiles.pop(t)

                def emit_gram(t):
                    sT_sb = sT_tiles.pop(t)
                    if t < SPLIT:
                        Sdst = S_ps
                    elif t < SPLIT2:
                        Sdst = S_ps_b
                    else:
                        Sdst = S_ps_c
                    for q in range(4):
                        a = sT_sb[:, 128 * q: 128 * (q + 1)]
                        nc.tensor.matmul(
                            Sdst[:], a, a,
                            start=(t in (0, SPLIT, SPLIT2) and q == 0),
                            stop=(t in (SPLIT - 1, SPLIT2 - 1, TTILES - 1)
                                  and q == 3))
                    if t == SPLIT - 1:
                        # evacuate partial gram (Gvq block + diag only),
                        # pre-scaled by temperature, and start its
                        # AllReduce while pass 1 continues.  All payload
                        # DMAs ride the SWDGE (gpsimd) queue so they never
                        # block the window loads on the HWDGE queues.
                        nc.vector.tensor_scalar(
                            out=Sa_sb[0:64, 0:64], in0=S_ps[0:64, 64:128],
                            scalar1=tb_sb[:], scalar2=None,
                            op0=mybir.AluOpType.mult)
                        nc.vector.tensor_mul(dtmp[:], S_ps[:], diag_msk[:])
                        with nc.allow_low_precision(
                                reason="bf16 collective payload"):
                            nc.vector.tensor_reduce(
                                diag_a[:], dtmp[:],
                                axis=mybir.AxisListType.X,
                                op=mybir.AluOpType.add)
                        nc.sync.dma_start(cc_in[0:64, 0:64],
                                          Sa_sb[0:64, 0:64])
                        nc.scalar.dma_start(cc_in[64:66, 0:64], diag_a[:])
                        nc.gpsimd.collective_compute(
                            "AllReduce", mybir.AluOpType.add,
                            replica_groups=RG,
                            ins=[cc_in.opt()], outs=[cc_out.opt()])
                    if t == SPLIT2 - 1:
                        nc.vector.tensor_scalar(
                            out=Sb_sb[:], in0=S_ps_b[0:64, 64:128],
                            scalar1=tb_sb[:], scalar2=None,
                            op0=mybir.AluOpType.mult)
                        nc.vector.tensor_mul(dtmp[:], S_ps_b[:], diag_msk[:])
                        with nc.allow_low_precision(
                                reason="bf16 collective payload"):
                            nc.vector.tensor_reduce(
                                diag_b[:], dtmp[:],
                                axis=mybir.AxisListType.X,
                                op=mybir.AluOpType.add)
                        nc.sync.dma_start(cc_in_b[0:64, 0:64], Sb_sb[:])
                        nc.scalar.dma_start(cc_in_b[64:66, 0:64], diag_b[:])
                        nc.gpsimd.collective_compute(
                            "AllReduce", mybir.AluOpType.add,
                            replica_groups=RG,
                            ins=[cc_in_b.opt()], outs=[cc_out_b.opt()])
                        # AR#1 readbacks go AFTER the AR#2 trigger on the
                        # SWDGE queue: they wait for AR#1 completion and
                        # must not delay AR#2's payload
                        nc.gpsimd.dma_start(gvq_a[:], cc_out[0:64, :])
                        nc.gpsimd.dma_start(sv_a[:], cc_out[64:65, :])
                        nc.gpsimd.dma_start(sq_a[:], cc_out[65:66, :])

                for w in range(NWIN + NWIN_T):
                    xd_t = pool_xd.tile([128, WIN], BF16)
                    xd3_t = pool_xd.tile([128, WIN], BF16)
                    if w < NWIN:
                        base = w * WINR * RS
                    else:
                        base = XLEN + (w - NWIN) * WINR * RS
                    if w <= 1:
                        # fine split so the first sub-tiles' taps land fast
                        nc.sync.dma_start(
                            xd_t[:, 0:1032], xd_d[:, base: base + 1032])
                        nc.scalar.dma_start(
                            xd3_t[:, 0:1032], xd3_d[:, base: base + 1032])
                        nc.sync.dma_start(
                            xd_t[:, 1032:WIN], xd_d[:, base + 1032: base + WIN])
                        nc.scalar.dma_start(
                            xd3_t[:, 1032:WIN],
                            xd3_d[:, base + 1032: base + WIN])
                    else:
                        nc.sync.dma_start(
                            xd_t[:], xd_d[:, base: base + WIN])
                        nc.scalar.dma_start(
                            xd3_t[:], xd3_d[:, base: base + WIN])
                    if w == 1:
                        # late weights, off the startup critical path
                        nc.sync.dma_start(wo_sb[:], wo_d[:])
                        nc.sync.dma_start(tp_sb[:], tp_d[:])
                    if w == 3:
                        # temperature broadcast column (needed at SPLIT-1)
                        tb_ps = pool_tb.tile([64, 1], F32)
                        nc.tensor.matmul(tb_ps[:], ones_sb[:], tp_sb[:],
                                         start=True, stop=True)
                        nc.vector.tensor_copy(tb_sb[:], tb_ps[:])
                    for j in range(SUBT):
                        t = SUBT * w + j
                        # x2 first: its ACT evacuation overlaps the conv MMs
                        x2_ps = pool_x2.tile([128, NT], F32)
                        nc.tensor.matmul(
                            x2_ps[:], w2_sb[:],
                            _rhs3(xd_t, 128, j, 1, 1),
                            start=True, stop=True)
                        x2_sb = pool_x2sb.tile([128, NT], F32)
                        nc.scalar.copy(x2_sb[:], x2_ps[:])
                        dw_ps = pool_dw.tile([128, NT], F32)
                        for p, (ky, kx) in enumerate(PAIR_TAPS):
                            rhs = _rhs3(xd_t, 128, j, ky, kx)
                            nc.tensor.matmul(
                                dw_ps[:],
                                wp_sb[:, 128 * p: 128 * (p + 1)],
                                rhs,
                                start=(p == 0), stop=False)
                        # xd3 low half = x+516 (tap (2,0)), high = x+2
                        # (tap (0,2)): one K=128 pair, then (2,2) single
                        nc.tensor.matmul(
                            dw_ps[:], ws_sb[:, 0:128],
                            _rhs3(xd3_t, 128, j, 0, 0),
                            start=False, stop=False)
                        nc.tensor.matmul(
                            dw_ps[:], ws_sb[:, 128:256],
                            _rhs3(xd3_t, 128, j, 0, 2),
                            start=False, stop=True)
                        # PE fills the wait for this tile's DVE mult with
                        # last tile's transposes and an older gram
                        if t >= 1:
                            emit_transpose(t - 1)
                        if t >= 2:
                            emit_gram(t - 2)
                        if t < NTILES:
                            dst = s_t[:, NT * t: NT * (t + 1)]
                        else:
                            sc_sb = pool_sc.tile([128, NT], BF16)
                            sc_tiles[t] = sc_sb
                            dst = sc_sb[:]
                        nc.vector.tensor_mul(dst, dw_ps[:], x2_sb[:])
                emit_transpose(TTILES - 1)
                emit_gram(TTILES - 2)
                emit_gram(TTILES - 1)

            # ---------------- local tail gram (no collective) ----------------
            gvq_c = pool_w.tile([64, 64], F32)
            nc.vector.tensor_scalar(
                out=gvq_c[:], in0=S_ps_c[0:64, 64:128],
                scalar1=tb_sb[:], scalar2=None, op0=mybir.AluOpType.mult)
            nc.vector.tensor_mul(dtmp[:], S_ps_c[:], diag_msk[:])

            # readback of AR#2
            gvq_b = pool_w.tile([64, 64], BF16)
            sv_b = pool_w.tile([1, 64], BF16)
            sq_b = pool_w.tile([1, 64], BF16)
            nc.sync.dma_start(gvq_b[:], cc_out_b[0:64, :])
            nc.scalar.dma_start(sv_b[:], cc_out_b[64:65, :])
            nc.scalar.dma_start(sq_b[:], cc_out_b[65:66, :])

            # ---------------- softmax + fused output weights ----------------
            with ExitStack() as p15:
                ps_sm = p15.enter_context(
                    tc.tile_pool(name="ps_sm", bufs=1, space="PSUM"))
                # local diag directly as a row: ones^T @ (S_c * I)
                dgT_ps = ps_sm.tile([1, 128], F32)
                nc.tensor.matmul(dgT_ps[:], ones_col[:], dtmp[:],
                                 start=True, stop=True)
                gvq_sb = pool_w.tile([64, 64], F32)     # [d, c] = v_d . q_c
                sv_s = pool_w.tile([1, 64], F32)
                sq_s = pool_w.tile([1, 64], F32)
                nc.vector.tensor_add(gvq_sb[:], gvq_a[:], gvq_b[:])
                nc.vector.tensor_add(gvq_sb[:], gvq_sb[:], gvq_c[:])
                nc.vector.tensor_add(sv_s[:], sv_a[:], sv_b[:])
                nc.vector.tensor_add(sv_s[:], sv_s[:], dgT_ps[0:1, 0:64])
                nc.vector.tensor_add(sq_s[:], sq_a[:], sq_b[:])
                nc.vector.tensor_add(sq_s[:], sq_s[:], dgT_ps[0:1, 64:128])
                # R[c,d] = rsqrt(sq[c] * sv[d]); temp is already in Gvq
                P_ps = ps_sm.tile([64, 64], F32)
                nc.tensor.matmul(P_ps[:], sq_s[:], sv_s[:],
                                 start=True, stop=True)
                Rm = pool_w.tile([64, 64], F32)
                nc.scalar.activation(
                    Rm[:], P_ps[:],
                    mybir.ActivationFunctionType.Abs_reciprocal_sqrt)
                # Gqv = Gvq^T
                gqv_ps = ps_sm.tile([64, 64], F32)
                nc.tensor.transpose(gqv_ps[:], gvq_sb[:],
                                    diag_msk[0:64, 0:64])
                # z = Gqv * R;  |z| <= temp so exp needs no max-shift
                z = pool_w.tile([64, 64], F32)
                nc.vector.tensor_mul(z[:], gqv_ps[:], Rm[:])
                e = pool_w.tile([64, 64], F32)
                sums = pool_w.tile([64, 1], F32)
                nc.scalar.activation(
                    e[:], z[:], mybir.ActivationFunctionType.Exp,
                    accum_out=sums[:])
                rs = pool_w.tile([64, 1], F32)
                nc.vector.reciprocal(rs[:], sums[:])
                attn = pool_w.tile([64, 64], F32)
                nc.vector.tensor_scalar(
                    out=attn[:], in0=e[:], scalar1=rs[:], scalar2=None,
                    op0=mybir.AluOpType.mult)
                # A2T = attn^T @ w_out^T  ->  [d, o], K=128-padded in bf16
                a2t_ps = ps_sm.tile([64, 64], F32)
                nc.tensor.matmul(a2t_ps[:], attn[:], wo_sb[:],
                                 start=True, stop=True)
                nc.vector.tensor_copy(a2t_bf[0:64, :], a2t_ps[:])

            # ---------------- pass 2: out = A2 @ v, streamed ----------------
            with ExitStack() as p2:
                ps_o = p2.enter_context(
                    tc.tile_pool(name="ps_o", bufs=7, space="PSUM"))
                ob_pool = p2.enter_context(tc.tile_pool(name="ob", bufs=6))
                # col-tiled pairs: partition half 0 covers output cols
                # [0, N/2), half 1 covers [N/2, N) -- the store DMAs are
                # then fully contiguous per channel row
                HB = N // 2
                BIG = 2048
                for T in range(HB // BIG):
                    ob_sb = ob_pool.tile([128, BIG], BF16)
                    for j in range(4):
                        k = (BIG // NT) * T + j
                        ps = ps_o.tile([128, NT], F32)
                        nc.tensor.matmul(
                            ps[0:64, :], a2t_bf[:],
                            s_t[:, NT * k: NT * (k + 1)],
                            start=True, stop=True)
                        nc.tensor.matmul(
                            ps[64:128, :], a2t_bf[:],
                            s_t[:, HB + NT * k: HB + NT * (k + 1)],
                            start=True, stop=True, tile_position=(0, 64))
                        if j % 2 == 0:
                            nc.scalar.copy(
                                ob_sb[:, NT * j: NT * (j + 1)], ps[:])
                        else:
                            nc.vector.tensor_copy(
                                ob_sb[:, NT * j: NT * (j + 1)], ps[:])
                    nc.sync.dma_start(
                        out_d[0:64, BIG * T: BIG * (T + 1)], ob_sb[0:64])
                    nc.scalar.dma_start(
                        out_d[0:64, HB + BIG * T: HB + BIG * (T + 1)],
                        ob_sb[64:128])

    nc.compile()
    return nc


def _get_nc():
    if "nc" not in _CACHE:
        _CACHE["nc"] = build_nc()
    return _CACHE["nc"]


def _dup_shift(xp):
    """[64, rows, 258] zero-padded block -> the xd / xd3 dup layouts."""
    xlen = xp.shape[1] * RS
    xpf = xp.reshape(64, xlen)
    xpf_pad = np.pad(xpf, ((0, 0), (0, 520)))
    xd = np.concatenate([xpf, xpf_pad[:, SHIFT:SHIFT + xlen]], axis=0)
    xd3 = np.concatenate(
        [xpf_pad[:, 516:516 + xlen], xpf_pad[:, 2:2 + xlen]], axis=0)
    return xd, xd3


def _prep_in_maps(x, w_in, w_dw, w_out, temperature):
    x = np.ascontiguousarray(x, dtype=np.float32)
    w_in = np.asarray(w_in, dtype=np.float32)
    w_dw = np.asarray(w_dw, dtype=np.float32)
    w_out = np.asarray(w_out, dtype=np.float32)
    temp = np.asarray(temperature, dtype=np.float32).reshape(1, 1)

    perm = np.concatenate([np.arange(64, 128), np.arange(0, 64)])
    W_in1 = w_in[:2 * C]          # [128, 64]
    W_in2 = w_in[2 * C:]          # [128, 64]
    wd = w_dw[:, 0]               # [128, 3, 3]

    import ml_dtypes
    bf = ml_dtypes.bfloat16

    wp = np.empty((128, 3 * 128), dtype=np.float32)
    for p, (ky, kx) in enumerate(PAIR_TAPS):
        wp[:64, 128 * p:128 * (p + 1)] = \
            (W_in1[perm] * wd[perm, ky, kx][:, None]).T
        wp[64:, 128 * p:128 * (p + 1)] = \
            (W_in1[perm] * wd[perm, ky + 1, kx + 1][:, None]).T
    # ws col block 0: K=128 pair [(2,0) @ rows 0-63, (0,2) @ rows 64-127]
    # (xd3 low half reads tap (2,0) at view (0,0), high half tap (0,2));
    # col block 1: single (2,2) @ rows 0-63 (xd3 low half, view (0,2))
    ws = np.zeros((128, 2 * 128), dtype=np.float32)
    ws[0:64, 0:128] = (W_in1[perm] * wd[perm, 2, 0][:, None]).T
    ws[64:128, 0:128] = (W_in1[perm] * wd[perm, 0, 2][:, None]).T
    ws[0:64, 128:256] = (W_in1[perm] * wd[perm, 2, 2][:, None]).T
    w2 = np.zeros((128, 128), dtype=np.float32)
    w2[0:64, :] = W_in2[perm].T
    wo = np.ascontiguousarray(w_out.T)              # [64, 64]
    wp = wp.astype(bf)
    ws = ws.astype(bf)
    w2 = w2.astype(bf)

    in_maps = []
    for core in range(8):
        sample, half = core // 2, core % 2
        h0 = half * 128
        xp = np.zeros((64, HROWS, RS), dtype=bf)
        lo, hi = max(h0 - 1, 0), min(h0 + 129, H)
        xp[:, lo - (h0 - 1): hi - (h0 - 1), 1:257] = x[sample, :, lo:hi, :]
        xd, xd3 = _dup_shift(xp)
        # partner tail: last 16 rows of the partner's half (+halo)
        p0 = (1 - half) * 128 + 128 - NWIN_T * WINR
        xq = np.zeros((64, TROWS, RS), dtype=bf)
        qlo, qhi = p0 - 1, min(p0 + NWIN_T * WINR + 1, H)
        xq[:, 0: qhi - qlo, 1:257] = x[sample, :, qlo:qhi, :]
        xdt, xd3t = _dup_shift(xq)
        in_maps.append({
            "xd": np.ascontiguousarray(
                np.concatenate([xd, xdt], axis=1)),
            "xd3": np.ascontiguousarray(
                np.concatenate([xd3, xd3t], axis=1)),
            "wp": wp, "ws": ws, "w2": w2, "wo": wo, "tp": temp,
        })
    return in_maps


def _assemble(results):
    out = np.empty((B, C, H, W), dtype=np.float32)
    for core in range(8):
        sample, half = core // 2, core % 2
        out[sample, :, half * 128: half * 128 + 128, :] = \
            results[core]["out"].astype(np.float32).reshape(C, 128, W)
    return out


def run(trace=False, trace_cores=None, **inputs):
    if trace:
        _install_ntff_hook()
    nc = _get_nc()
    in_maps = _prep_in_maps(**inputs)
    res = run_bass_kernel_spmd(nc, in_maps, core_ids=list(range(8)),
                               trace=trace, trace_cores=trace_cores)
    if trace and res.mean_exec_time_ns:
        print(f"mean exec {res.mean_exec_time_ns/1000:.1f}us, "
              f"max core {res.max_exec_time_core_id}")
    return _assemble(res.results), res.exec_time_ns


def kernel(**inputs) -> np.ndarray:
    out, _ = run(trace=False, **inputs)
    return out


# revision 60
# speedup vs baseline: 1.0679x; 1.0679x over previous
"""Trainium2 Bass kernel for nn_Attention_28802050687173.

Channel-attention block: 1x1 conv (c->4c), depthwise 3x3, gating multiply,
L2-normalized channel gram + softmax, attn @ v, 1x1 conv out.

Sharding: 8 cores = (sample, H-half).  Each core processes 128 rows x 256 cols
of one sample (n_loc = 32768 pixels).  The depthwise conv is folded into the
input projection: dw = sum_j (w_dw[:,j] * W_in1) @ x_shift_j, so the whole
front end is 7 matmuls per tile over a zero-padded, duplicated+shifted copy of
x built host-side.  The channel gram S = [v;q][v;q]^T is accumulated on-chip
(PE transposes + bf16 matmuls).  To keep the gram AllReduce off the critical
path, each core redundantly computes the gram contribution of its PARTNER's
last 8 tiles (2 extra input windows): the collectives then only cover tiles
0..55 and complete under the tail compute.  Softmax and the fused
(w_out @ attn) @ v output projection follow, stored in bf16.
"""
import numpy as np

import concourse.bass as bass
import concourse.mybir as mybir
import concourse.tile as tile
from concourse import bacc
from concourse.bass_utils import run_bass_kernel_spmd
from concourse.masks import make_identity

F32 = mybir.dt.float32
F32R = mybir.dt.float32r
BF16 = mybir.dt.bfloat16


def _install_ntff_hook():
    """The container's antenv stub lacks axon_hooks, so bass_utils'
    trace=True path can't find the NTFF profile hook the axon .so
    provides.  Recreate the hook (same ctypes ABI trn_agent_boot uses)
    and inject an antenv.axon_hooks module exposing it."""
    import sys
    import contextlib
    import ctypes
    if "antenv.axon_hooks" in sys.modules:
        return
    so_path = "/opt/axon/libaxon_pjrt.so"
    try:
        lib = ctypes.CDLL(so_path)
    except OSError:
        return
    if not hasattr(lib, "axon_start_nrt_profile"):
        return
    lib.axon_start_nrt_profile.argtypes = [
        ctypes.POINTER(ctypes.c_int64), ctypes.c_size_t]
    lib.axon_start_nrt_profile.restype = ctypes.c_int64
    lib.axon_stop_nrt_profile.argtypes = [ctypes.c_char_p]
    lib.axon_stop_nrt_profile.restype = ctypes.c_int64

    @contextlib.contextmanager
    def _hook(output_dir, device_ids):
        import jax
        jax.devices()
        if device_ids:
            ids = (ctypes.c_int64 * len(device_ids))(*device_ids)
            rc = lib.axon_start_nrt_profile(ids, len(device_ids))
        else:
            rc = lib.axon_start_nrt_profile(None, 0)
        if rc != 0:
            raise RuntimeError(f"axon_start_nrt_profile rc={rc}")
        try:
            yield
        finally:
            n = lib.axon_stop_nrt_profile(str(output_dir).encode())
            if n < 0:
                raise RuntimeError(f"axon_stop_nrt_profile rc={n}")

    import types
    mod = types.ModuleType("antenv.axon_hooks")
    mod._hook = _hook
    mod.get_axon_ntff_profile_hook = lambda: mod._hook
    mod.set_axon_ntff_profile_hook = lambda h: setattr(mod, "_hook", h)
    sys.modules["antenv.axon_hooks"] = mod
    try:
        import antenv
        antenv.axon_hooks = mod
    except ImportError:
        pass

B, C, H, W = 4, 64, 256, 256
RS = 258                     # zero-padded row stride
HROWS = 130                  # 128 output rows + 1 halo row each side
XLEN = HROWS * RS            # 33540 elements per channel per core
TROWS = 18                   # partner-tail block: 16 output rows + halo
XT = TROWS * RS              # 4644
XLT = XLEN + XT
SHIFT = 259                  # dup-half shift: tap (ky,kx) -> (ky+1,kx+1)
N = 128 * 256                # 32768 outputs per core
NT = 512                     # matmul/psum tile (2 output rows)
WINR = 8                     # output rows per DMA window
WIN = (WINR + 2) * RS        # 2580 elements per window
NWIN = 128 // WINR           # 16 windows
NWIN_T = 2                   # partner-tail windows
SUBT = WINR // 2             # 4 sub-tiles per window
NTILES = N // NT             # 64 own tiles
TTILES = NTILES + NWIN_T * SUBT   # 72 incl. partner tail
SPLIT = 32                   # tiles [0, SPLIT) -> S_a (AllReduce #1)
SPLIT2 = 56                  # tiles [SPLIT, SPLIT2) -> S_b (AllReduce #2)
PAIR_TAPS = [(0, 0), (0, 1), (1, 0)]     # (ky,kx); partner = (ky+1,kx+1)
RG = [[0, 1], [2, 3], [4, 5], [6, 7]]    # AllReduce pairs (same sample)

_CACHE = {}


def _rhs3(xd_t, parts, j, ky, kx, p0=0):
    """[parts, 2, 256] view: output sub-tile j, tap (ky, kx)."""
    v = xd_t[p0:p0 + parts, :].rearrange("p (r c) -> p r c", r=WINR + 2, c=RS)
    return v[:, 2 * j + ky: 2 * j + ky + 2, kx: kx + 256]


def build_nc():
    nc = bacc.Bacc("TRN2", target_bir_lowering=False, debug=False, num_devices=8)

    xd_d = nc.dram_tensor("xd", [128, XLT], BF16, kind="ExternalInput")
    xd3_d = nc.dram_tensor("xd3", [128, XLT], BF16, kind="ExternalInput")
    wp_d = nc.dram_tensor("wp", [128, 3 * 128], BF16, kind="ExternalInput")
    ws_d = nc.dram_tensor("ws", [128, 2 * 128], BF16, kind="ExternalInput")
    w2_d = nc.dram_tensor("w2", [128, 128], BF16, kind="ExternalInput")
    wo_d = nc.dram_tensor("wo", [64, 64], F32, kind="ExternalInput")
    tp_d = nc.dram_tensor("tp", [1, 1], F32, kind="ExternalInput")
    out_d = nc.dram_tensor("out", [64, N], BF16, kind="ExternalOutput")

    with tile.TileContext(nc) as tc:
        from contextlib import ExitStack
        with ExitStack() as outer:
            pool_w = outer.enter_context(tc.tile_pool(name="wts", bufs=1))
            pool_s = outer.enter_context(tc.tile_pool(name="sbuf_s", bufs=1))
            pool_ps_S = outer.enter_context(
                tc.tile_pool(name="ps_S", bufs=1, space="PSUM"))
            pool_dram = outer.enter_context(
                tc.tile_pool(name="dram", bufs=1, space="DRAM"))

            # persistent tiles
            wp_sb = pool_w.tile([128, 3 * 128], BF16)
            ws_sb = pool_w.tile([128, 2 * 128], BF16)
            w2_sb = pool_w.tile([128, 128], BF16)
            wo_sb = pool_w.tile([64, 64], F32)
            tp_sb = pool_w.tile([1, 1], F32)
            id_bf = pool_w.tile([128, 128], BF16)
            s_t = pool_s.tile([128, N], BF16)
            S_all = pool_ps_S.tile([128, 384], F32)
            S_ps = S_all[:, 0:128]
            S_ps_b = S_all[:, 128:256]
            S_ps_c = S_all[:, 256:384]
            cc_in = pool_dram.tile([66, 64], BF16)
            cc_out = pool_dram.tile([66, 64], BF16)
            cc_in_b = pool_dram.tile([66, 64], BF16)
            cc_out_b = pool_dram.tile([66, 64], BF16)
            dmy_in = pool_dram.tile([1, 16], F32)
            dmy_out = pool_dram.tile([1, 16], F32)
            warm_d = pool_dram.tile([1, 16], F32)

            # tiny dummy AllReduce: pays the one-time mesh-algo init on the
            # CC core and absorbs inter-core launch skew while pass 1 runs.
            # Its payload rides HWDGE so the trigger fires within ~5us; a
            # separate throwaway SWDGE DMA pays the ~30us software-DGE
            # cold-start in the background before the real payloads need it.
            dmy_sb = pool_w.tile([1, 16], F32)
            nc.gpsimd.memset(dmy_sb[:], 1.0)
            nc.sync.dma_start(dmy_in[:], dmy_sb[:])
            nc.gpsimd.collective_compute(
                "AllReduce", mybir.AluOpType.add, replica_groups=RG,
                ins=[dmy_in.opt()], outs=[dmy_out.opt()])
            nc.gpsimd.dma_start(warm_d[:], dmy_sb[:])
            # weights ride the scalar queue so the sync queue starts window
            # 0 immediately; wo/tp (needed late) load inside the loop
            nc.scalar.dma_start(wp_sb[:], wp_d[:])
            nc.scalar.dma_start(w2_sb[:], w2_d[:])
            nc.scalar.dma_start(ws_sb[:], ws_d[:])
            make_identity(nc, id_bf[:])
            # preload ACT table sets (exp, abs_rsqrt) so the softmax phase
            # does not pay the ~2.7us-per-set load inside the collective gap
            scr_a = pool_w.tile([1, 1], F32)
            scr_b = pool_w.tile([1, 1], F32)
            nc.scalar.activation(scr_a[:], dmy_sb[0:1, 0:1],
                                 mybir.ActivationFunctionType.Exp)
            nc.scalar.activation(scr_b[:], scr_a[:],
                                 mybir.ActivationFunctionType.Abs_reciprocal_sqrt)
            ones_sb = pool_w.tile([1, 64], F32)
            nc.gpsimd.memset(ones_sb[:], 1.0)
            ones_col = pool_w.tile([128, 1], F32)
            nc.gpsimd.memset(ones_col[:], 1.0)
            tb_sb = pool_w.tile([64, 1], F32)   # temp broadcast column
            # constant f32 diag mask (expanded from bf16 identity)
            diag_msk = pool_w.tile([128, 128], F32)
            nc.scalar.copy(diag_msk[:], id_bf[:])
            # pass-2 weights buffer, zero-padded to K=128
            a2t_bf = pool_w.tile([128, 64], BF16)
            nc.gpsimd.memset(a2t_bf[:], 0.0)

            # ---------------- pass 1: conv front-end + gram ----------------
            # bf16 collective payloads: halves the fabric transfer time of
            # the AllReduces; the gram entries only feed softmax logits so
            # the 0.4% rounding is well inside the error budget
            Sa_sb = pool_w.tile([128, 128], BF16)
            dtmp = pool_w.tile([128, 128], F32)
            diag_a = pool_w.tile([128, 1], BF16)
            diag_b = pool_w.tile([128, 1], BF16)
            diag_c = pool_w.tile([128, 1], F32)
            Sb_sb = pool_w.tile([64, 64], BF16)
            gvq_a = pool_w.tile([64, 64], BF16)  # AR#1 result readback
            sv_a = pool_w.tile([1, 64], BF16)
            sq_a = pool_w.tile([1, 64], BF16)
            # warm the PE HAM before pass 1: a dense burst of dummy
            # matmuls with (almost) no dependencies that runs during the
            # initial DMA waits
            with tc.tile_pool(name="ps_w0", bufs=1, space="PSUM") as pw0:
                warm0 = pw0.tile([128, 128], F32)
                for _ in range(12):
                    nc.tensor.matmul(warm0[:], wp_sb[:, 0:128],
                                     wp_sb[:, 0:128], start=True, stop=True)

            with ExitStack() as p1:
                pool_xd = p1.enter_context(tc.tile_pool(name="xd", bufs=8))
                pool_tb = p1.enter_context(
                    tc.tile_pool(name="ps_tb", bufs=1, space="PSUM"))
                pool_dw = p1.enter_context(
                    tc.tile_pool(name="ps_dw", bufs=3, space="PSUM"))
                pool_x2 = p1.enter_context(
                    tc.tile_pool(name="ps_x2", bufs=1, space="PSUM"))
                pool_tr = p1.enter_context(
                    tc.tile_pool(name="ps_tr", bufs=2, space="PSUM"))
                pool_x2sb = p1.enter_context(tc.tile_pool(name="x2sb", bufs=4))
                pool_st = p1.enter_context(tc.tile_pool(name="stsb", bufs=6))
                pool_sc = p1.enter_context(tc.tile_pool(name="scsb", bufs=3))

                sT_tiles = {}
                sc_tiles = {}

                def src_of(t):
                    if t < NTILES:
                        return s_t[:, NT * t: NT * (t + 1)]
                    return sc_tiles[t][:]

                def emit_transpose(t):
                    src = src_of(t)
                    tr_ps = pool_tr.tile([128, NT], BF16)
                    for q in range(4):
                        nc.tensor.transpose(
                            tr_ps[:, 128 * q: 128 * (q + 1)],
                            src[:, 128 * q: 128 * (q + 1)],
                            id_bf[:])
                    sT_sb = pool_st.tile([128, NT], BF16)
                    nc.vector.tensor_copy(sT_sb[:], tr_ps[:])
                    sT_tiles[t] = sT_sb
                    if t >= NTILES:
                        sc_tiles.pop(t)

                def emit_gram(t):
                    sT_sb = sT_tiles.pop(t)
                    if t < SPLIT:
                        Sdst = S_ps
                    elif t < SPLIT2:
                        Sdst = S_ps_b
                    else:
                        Sdst = S_ps_c
                    for q in range(4):
                        a = sT_sb[:, 128 * q: 128 * (q + 1)]
                        nc.tensor.matmul(
                            Sdst[:], a, a,
                            start=(t in (0, SPLIT, SPLIT2) and q == 0),
                            stop=(t in (SPLIT - 1, SPLIT2 - 1, TTILES - 1)
                                  and q == 3))
                    if t == SPLIT - 1:
                        # evacuate partial gram (Gvq block + diag only),
                        # pre-scaled by temperature, and start its
                        # AllReduce while pass 1 continues.  All payload
                        # DMAs ride the SWDGE (gpsimd) queue so they never
                        # block the window loads on the HWDGE queues.
                        nc.vector.tensor_scalar(
                            out=Sa_sb[0:64, 0:64], in0=S_ps[0:64, 64:128],
                            scalar1=tb_sb[:], scalar2=None,
                            op0=mybir.AluOpType.mult)
                        nc.vector.tensor_mul(dtmp[:], S_ps[:], diag_msk[:])
                        with nc.allow_low_precision(
                                reason="bf16 collective payload"):
                            nc.vector.tensor_reduce(
                                diag_a[:], dtmp[:],
                                axis=mybir.AxisListType.X,
                                op=mybir.AluOpType.add)
                        nc.gpsimd.dma_start(cc_in[0:64, 0:64],
                                            Sa_sb[0:64, 0:64])
                        nc.gpsimd.dma_start(cc_in[64:66, 0:64], diag_a[:])
                        nc.gpsimd.collective_compute(
                            "AllReduce", mybir.AluOpType.add,
                            replica_groups=RG,
                            ins=[cc_in.opt()], outs=[cc_out.opt()])
                    if t == SPLIT2 - 1:
                        nc.vector.tensor_scalar(
                            out=Sb_sb[:], in0=S_ps_b[0:64, 64:128],
                            scalar1=tb_sb[:], scalar2=None,
                            op0=mybir.AluOpType.mult)
                        nc.vector.tensor_mul(dtmp[:], S_ps_b[:], diag_msk[:])
                        with nc.allow_low_precision(
                                reason="bf16 collective payload"):
                            nc.vector.tensor_reduce(
                                diag_b[:], dtmp[:],
                                axis=mybir.AxisListType.X,
                                op=mybir.AluOpType.add)
                        nc.gpsimd.dma_start(cc_in_b[0:64, 0:64], Sb_sb[:])
                        nc.gpsimd.dma_start(cc_in_b[64:66, 0:64], diag_b[:])
                        nc.gpsimd.collective_compute(
                            "AllReduce", mybir.AluOpType.add,
                            replica_groups=RG,
                            ins=[cc_in_b.opt()], outs=[cc_out_b.opt()])
                        # AR#1 readbacks go AFTER the AR#2 trigger on the
                        # SWDGE queue: they wait for AR#1 completion and
                        # must not delay AR#2's payload
                        nc.gpsimd.dma_start(gvq_a[:], cc_out[0:64, :])
                        nc.gpsimd.dma_start(sv_a[:], cc_out[64:65, :])
                        nc.gpsimd.dma_start(sq_a[:], cc_out[65:66, :])

                for w in range(NWIN + NWIN_T):
                    xd_t = pool_xd.tile([128, WIN], BF16)
                    xd3_t = pool_xd.tile([128, WIN], BF16)
                    if w < NWIN:
                        base = w * WINR * RS
                    else:
                        base = XLEN + (w - NWIN) * WINR * RS
                    if w <= 1:
                        # fine split so the first sub-tiles' taps land fast
                        nc.sync.dma_start(
                            xd_t[:, 0:1032], xd_d[:, base: base + 1032])
                        nc.scalar.dma_start(
                            xd3_t[:, 0:1032], xd3_d[:, base: base + 1032])
                        nc.sync.dma_start(
                            xd_t[:, 1032:WIN], xd_d[:, base + 1032: base + WIN])
                        nc.scalar.dma_start(
                            xd3_t[:, 1032:WIN],
                            xd3_d[:, base + 1032: base + WIN])
                    else:
                        nc.sync.dma_start(
                            xd_t[:], xd_d[:, base: base + WIN])
                        nc.scalar.dma_start(
                            xd3_t[:], xd3_d[:, base: base + WIN])
                    if w == 1:
                        # late weights, off the startup critical path
                        nc.sync.dma_start(wo_sb[:], wo_d[:])
                        nc.sync.dma_start(tp_sb[:], tp_d[:])
                    if w == 3:
                        # temperature broadcast column (needed at SPLIT-1)
                        tb_ps = pool_tb.tile([64, 1], F32)
                        nc.tensor.matmul(tb_ps[:], ones_sb[:], tp_sb[:],
                                         start=True, stop=True)
                        nc.vector.tensor_copy(tb_sb[:], tb_ps[:])
                    for j in range(SUBT):
                        t = SUBT * w + j
                        # x2 first: its ACT evacuation overlaps the conv MMs
                        x2_ps = pool_x2.tile([128, NT], F32)
                        nc.tensor.matmul(
                            x2_ps[:], w2_sb[:],
                            _rhs3(xd_t, 128, j, 1, 1),
                            start=True, stop=True)
                        x2_sb = pool_x2sb.tile([128, NT], F32)
                        nc.scalar.copy(x2_sb[:], x2_ps[:])
                        dw_ps = pool_dw.tile([128, NT], F32)
                        for p, (ky, kx) in enumerate(PAIR_TAPS):
                            rhs = _rhs3(xd_t, 128, j, ky, kx)
                            nc.tensor.matmul(
                                dw_ps[:],
                                wp_sb[:, 128 * p: 128 * (p + 1)],
                                rhs,
                                start=(p == 0), stop=False)
                        # xd3 low half = x+516 (tap (2,0)), high = x+2
                        # (tap (0,2)): one K=128 pair, then (2,2) single
                        nc.tensor.matmul(
                            dw_ps[:], ws_sb[:, 0:128],
                            _rhs3(xd3_t, 128, j, 0, 0),
                            start=False, stop=False)
                        nc.tensor.matmul(
                            dw_ps[:], ws_sb[:, 128:256],
                            _rhs3(xd3_t, 128, j, 0, 2),
                            start=False, stop=True)
                        # PE fills the wait for this tile's DVE mult with
                        # last tile's transposes and an older gram
                        if t >= 1:
                            emit_transpose(t - 1)
                        if t >= 2:
                            emit_gram(t - 2)
                        if t < NTILES:
                            dst = s_t[:, NT * t: NT * (t + 1)]
                        else:
                            sc_sb = pool_sc.tile([128, NT], BF16)
                            sc_tiles[t] = sc_sb
                            dst = sc_sb[:]
                        nc.vector.tensor_mul(dst, dw_ps[:], x2_sb[:])
                emit_transpose(TTILES - 1)
                emit_gram(TTILES - 2)
                emit_gram(TTILES - 1)

            # ---------------- local tail gram (no collective) ----------------
            gvq_c = pool_w.tile([64, 64], F32)
            nc.vector.tensor_scalar(
                out=gvq_c[:], in0=S_ps_c[0:64, 64:128],
                scalar1=tb_sb[:], scalar2=None, op0=mybir.AluOpType.mult)
            nc.vector.tensor_mul(dtmp[:], S_ps_c[:], diag_msk[:])

            # readback of AR#2
            gvq_b = pool_w.tile([64, 64], BF16)
            sv_b = pool_w.tile([1, 64], BF16)
            sq_b = pool_w.tile([1, 64], BF16)
            nc.sync.dma_start(gvq_b[:], cc_out_b[0:64, :])
            nc.scalar.dma_start(sv_b[:], cc_out_b[64:65, :])
            nc.scalar.dma_start(sq_b[:], cc_out_b[65:66, :])

            # ---------------- softmax + fused output weights ----------------
            with ExitStack() as p15:
                ps_sm = p15.enter_context(
                    tc.tile_pool(name="ps_sm", bufs=1, space="PSUM"))
                # local diag directly as a row: ones^T @ (S_c * I)
                dgT_ps = ps_sm.tile([1, 128], F32)
                nc.tensor.matmul(dgT_ps[:], ones_col[:], dtmp[:],
                                 start=True, stop=True)
                gvq_sb = pool_w.tile([64, 64], F32)     # [d, c] = v_d . q_c
                sv_s = pool_w.tile([1, 64], F32)
                sq_s = pool_w.tile([1, 64], F32)
                nc.vector.tensor_add(gvq_sb[:], gvq_a[:], gvq_b[:])
                nc.vector.tensor_add(gvq_sb[:], gvq_sb[:], gvq_c[:])
                nc.vector.tensor_add(sv_s[:], sv_a[:], sv_b[:])
                nc.vector.tensor_add(sv_s[:], sv_s[:], dgT_ps[0:1, 0:64])
                nc.vector.tensor_add(sq_s[:], sq_a[:], sq_b[:])
                nc.vector.tensor_add(sq_s[:], sq_s[:], dgT_ps[0:1, 64:128])
                # R[c,d] = rsqrt(sq[c] * sv[d]); temp is already in Gvq
                P_ps = ps_sm.tile([64, 64], F32)
                nc.tensor.matmul(P_ps[:], sq_s[:], sv_s[:],
                                 start=True, stop=True)
                Rm = pool_w.tile([64, 64], F32)
                nc.scalar.activation(
                    Rm[:], P_ps[:],
                    mybir.ActivationFunctionType.Abs_reciprocal_sqrt)
                # Gqv = Gvq^T
                gqv_ps = ps_sm.tile([64, 64], F32)
                nc.tensor.transpose(gqv_ps[:], gvq_sb[:],
                                    diag_msk[0:64, 0:64])
                # z = Gqv * R;  |z| <= temp so exp needs no max-shift
                z = pool_w.tile([64, 64], F32)
                nc.vector.tensor_mul(z[:], gqv_ps[:], Rm[:])
                e = pool_w.tile([64, 64], F32)
                sums = pool_w.tile([64, 1], F32)
                nc.scalar.activation(
                    e[:], z[:], mybir.ActivationFunctionType.Exp,
                    accum_out=sums[:])
                rs = pool_w.tile([64, 1], F32)
                nc.vector.reciprocal(rs[:], sums[:])
                attn = pool_w.tile([64, 64], F32)
                nc.vector.tensor_scalar(
                    out=attn[:], in0=e[:], scalar1=rs[:], scalar2=None,
                    op0=mybir.AluOpType.mult)
                # A2T = attn^T @ w_out^T  ->  [d, o], K=128-padded in bf16
                a2t_ps = ps_sm.tile([64, 64], F32)
                nc.tensor.matmul(a2t_ps[:], attn[:], wo_sb[:],
                                 start=True, stop=True)
                nc.vector.tensor_copy(a2t_bf[0:64, :], a2t_ps[:])

            # ---------------- pass 2: out = A2 @ v, streamed ----------------
            with ExitStack() as p2:
                ps_o = p2.enter_context(
                    tc.tile_pool(name="ps_o", bufs=7, space="PSUM"))
                ob_pool = p2.enter_context(tc.tile_pool(name="ob", bufs=6))
                # col-tiled pairs: partition half 0 covers output cols
                # [0, N/2), half 1 covers [N/2, N) -- the store DMAs are
                # then fully contiguous per channel row
                HB = N // 2
                BIG = 2048
                for T in range(HB // BIG):
                    ob_sb = ob_pool.tile([128, BIG], BF16)
                    for j in range(4):
                        k = (BIG // NT) * T + j
                        ps = ps_o.tile([128, NT], F32)
                        nc.tensor.matmul(
                            ps[0:64, :], a2t_bf[:],
                            s_t[:, NT * k: NT * (k + 1)],
                            start=True, stop=True)
                        nc.tensor.matmul(
                            ps[64:128, :], a2t_bf[:],
                            s_t[:, HB + NT * k: HB + NT * (k + 1)],
                            start=True, stop=True, tile_position=(0, 64))
                        if j % 2 == 0:
                            nc.scalar.copy(
                                ob_sb[:, NT * j: NT * (j + 1)], ps[:])
                        else:
                            nc.vector.tensor_copy(
                                ob_sb[:, NT * j: NT * (j + 1)], ps[:])
                    nc.sync.dma_start(
                        out_d[0:64, BIG * T: BIG * (T + 1)], ob_sb[0:64])
                    nc.scalar.dma_start(
                        out_d[0:64, HB + BIG * T: HB + BIG * (T + 1)],
                        ob_sb[64:128])

    nc.compile()
    return nc


def _get_nc():
    if "nc" not in _CACHE:
        _CACHE["nc"] = build_nc()
    return _CACHE["nc"]


def _dup_shift(xp):
    """[64, rows, 258] zero-padded block -> the xd / xd3 dup layouts."""
    xlen = xp.shape[1] * RS
    xpf = xp.reshape(64, xlen)
    xpf_pad = np.pad(xpf, ((0, 0), (0, 520)))
    xd = np.concatenate([xpf, xpf_pad[:, SHIFT:SHIFT + xlen]], axis=0)
    xd3 = np.concatenate(
        [xpf_pad[:, 516:516 + xlen], xpf_pad[:, 2:2 + xlen]], axis=0)
    return xd, xd3


def _prep_in_maps(x, w_in, w_dw, w_out, temperature):
    x = np.ascontiguousarray(x, dtype=np.float32)
    w_in = np.asarray(w_in, dtype=np.float32)
    w_dw = np.asarray(w_dw, dtype=np.float32)
    w_out = np.asarray(w_out, dtype=np.float32)
    temp = np.asarray(temperature, dtype=np.float32).reshape(1, 1)

    perm = np.concatenate([np.arange(64, 128), np.arange(0, 64)])
    W_in1 = w_in[:2 * C]          # [128, 64]
    W_in2 = w_in[2 * C:]          # [128, 64]
    wd = w_dw[:, 0]               # [128, 3, 3]

    import ml_dtypes
    bf = ml_dtypes.bfloat16

    wp = np.empty((128, 3 * 128), dtype=np.float32)
    for p, (ky, kx) in enumerate(PAIR_TAPS):
        wp[:64, 128 * p:128 * (p + 1)] = \
            (W_in1[perm] * wd[perm, ky, kx][:, None]).T
        wp[64:, 128 * p:128 * (p + 1)] = \
            (W_in1[perm] * wd[perm, ky + 1, kx + 1][:, None]).T
    # ws col block 0: K=128 pair [(2,0) @ rows 0-63, (0,2) @ rows 64-127]
    # (xd3 low half reads tap (2,0) at view (0,0), high half tap (0,2));
    # col block 1: single (2,2) @ rows 0-63 (xd3 low half, view (0,2))
    ws = np.zeros((128, 2 * 128), dtype=np.float32)
    ws[0:64, 0:128] = (W_in1[perm] * wd[perm, 2, 0][:, None]).T
    ws[64:128, 0:128] = (W_in1[perm] * wd[perm, 0, 2][:, None]).T
    ws[0:64, 128:256] = (W_in1[perm] * wd[perm, 2, 2][:, None]).T
    w2 = np.zeros((128, 128), dtype=np.float32)
    w2[0:64, :] = W_in2[perm].T
    wo = np.ascontiguousarray(w_out.T)              # [64, 64]
    wp = wp.astype(bf)
    ws = ws.astype(bf)
    w2 = w2.astype(bf)

    in_maps = []
    for core in range(8):
        sample, half = core // 2, core % 2
        h0 = half * 128
        xp = np.zeros((64, HROWS, RS), dtype=bf)
        lo, hi = max(h0 - 1, 0), min(h0 + 129, H)
        xp[:, lo - (h0 - 1): hi - (h0 - 1), 1:257] = x[sample, :, lo:hi, :]
        xd, xd3 = _dup_shift(xp)
        # partner tail: last 16 rows of the partner's half (+halo)
        p0 = (1 - half) * 128 + 128 - NWIN_T * WINR
        xq = np.zeros((64, TROWS, RS), dtype=bf)
        qlo, qhi = p0 - 1, min(p0 + NWIN_T * WINR + 1, H)
        xq[:, 0: qhi - qlo, 1:257] = x[sample, :, qlo:qhi, :]
        xdt, xd3t = _dup_shift(xq)
        in_maps.append({
            "xd": np.ascontiguousarray(
                np.concatenate([xd, xdt], axis=1)),
            "xd3": np.ascontiguousarray(
                np.concatenate([xd3, xd3t], axis=1)),
            "wp": wp, "ws": ws, "w2": w2, "wo": wo, "tp": temp,
        })
    return in_maps


def _assemble(results):
    out = np.empty((B, C, H, W), dtype=np.float32)
    for core in range(8):
        sample, half = core // 2, core % 2
        out[sample, :, half * 128: half * 128 + 128, :] = \
            results[core]["out"].astype(np.float32).reshape(C, 128, W)
    return out


def run(trace=False, trace_cores=None, **inputs):
    if trace:
        _install_ntff_hook()
    nc = _get_nc()
    in_maps = _prep_in_maps(**inputs)
    res = run_bass_kernel_spmd(nc, in_maps, core_ids=list(range(8)),
                               trace=trace, trace_cores=trace_cores)
    if trace and res.mean_exec_time_ns:
        print(f"mean exec {res.mean_exec_time_ns/1000:.1f}us, "
              f"max core {res.max_exec_time_core_id}")
    return _assemble(res.results), res.exec_time_ns


def kernel(**inputs) -> np.ndarray:
    out, _ = run(trace=False, **inputs)
    return out


# revision 65
# speedup vs baseline: 1.1199x; 1.0487x over previous
"""Trainium2 Bass kernel for nn_Attention_28802050687173.

Channel-attention block: 1x1 conv (c->4c), depthwise 3x3, gating multiply,
L2-normalized channel gram + softmax, attn @ v, 1x1 conv out.

Sharding: 8 cores = (sample, H-half).  Each core processes 128 rows x 256 cols
of one sample (n_loc = 32768 pixels).  The depthwise conv is folded into the
input projection: dw = sum_j (w_dw[:,j] * W_in1) @ x_shift_j, so the whole
front end is 7 matmuls per tile over a zero-padded, duplicated+shifted copy of
x built host-side.  The channel gram S = [v;q][v;q]^T is accumulated on-chip
(PE transposes + bf16 matmuls).  To keep the gram AllReduce off the critical
path, each core redundantly computes the gram contribution of its PARTNER's
last 8 tiles (2 extra input windows): the collectives then only cover tiles
0..55 and complete under the tail compute.  Softmax and the fused
(w_out @ attn) @ v output projection follow, stored in bf16.
"""
import numpy as np

import concourse.bass as bass
import concourse.mybir as mybir
import concourse.tile as tile
from concourse import bacc
from concourse.bass_utils import run_bass_kernel_spmd
from concourse.masks import make_identity

F32 = mybir.dt.float32
F32R = mybir.dt.float32r
BF16 = mybir.dt.bfloat16


def _install_ntff_hook():
    """The container's antenv stub lacks axon_hooks, so bass_utils'
    trace=True path can't find the NTFF profile hook the axon .so
    provides.  Recreate the hook (same ctypes ABI trn_agent_boot uses)
    and inject an antenv.axon_hooks module exposing it."""
    import sys
    import contextlib
    import ctypes
    if "antenv.axon_hooks" in sys.modules:
        return
    so_path = "/opt/axon/libaxon_pjrt.so"
    try:
        lib = ctypes.CDLL(so_path)
    except OSError:
        return
    if not hasattr(lib, "axon_start_nrt_profile"):
        return
    lib.axon_start_nrt_profile.argtypes = [
        ctypes.POINTER(ctypes.c_int64), ctypes.c_size_t]
    lib.axon_start_nrt_profile.restype = ctypes.c_int64
    lib.axon_stop_nrt_profile.argtypes = [ctypes.c_char_p]
    lib.axon_stop_nrt_profile.restype = ctypes.c_int64

    @contextlib.contextmanager
    def _hook(output_dir, device_ids):
        import jax
        jax.devices()
        if device_ids:
            ids = (ctypes.c_int64 * len(device_ids))(*device_ids)
            rc = lib.axon_start_nrt_profile(ids, len(device_ids))
        else:
            rc = lib.axon_start_nrt_profile(None, 0)
        if rc != 0:
            raise RuntimeError(f"axon_start_nrt_profile rc={rc}")
        try:
            yield
        finally:
            n = lib.axon_stop_nrt_profile(str(output_dir).encode())
            if n < 0:
                raise RuntimeError(f"axon_stop_nrt_profile rc={n}")

    import types
    mod = types.ModuleType("antenv.axon_hooks")
    mod._hook = _hook
    mod.get_axon_ntff_profile_hook = lambda: mod._hook
    mod.set_axon_ntff_profile_hook = lambda h: setattr(mod, "_hook", h)
    sys.modules["antenv.axon_hooks"] = mod
    try:
        import antenv
        antenv.axon_hooks = mod
    except ImportError:
        pass

B, C, H, W = 4, 64, 256, 256
RS = 258                     # zero-padded row stride
HROWS = 130                  # 128 output rows + 1 halo row each side
XLEN = HROWS * RS            # 33540 elements per channel per core
TROWS = 18                   # partner-tail block: 16 output rows + halo
XT = TROWS * RS              # 4644
XLT = XLEN + XT
SHIFT = 259                  # dup-half shift: tap (ky,kx) -> (ky+1,kx+1)
N = 128 * 256                # 32768 outputs per core
NT = 512                     # matmul/psum tile (2 output rows)
WINR = 8                     # output rows per DMA window
WIN = (WINR + 2) * RS        # 2580 elements per window
NWIN = 128 // WINR           # 16 windows
NWIN_T = 2                   # partner-tail windows
SUBT = WINR // 2             # 4 sub-tiles per window
NTILES = N // NT             # 64 own tiles
TTILES = NTILES + NWIN_T * SUBT   # 72 incl. partner tail
SPLIT = 32                   # tiles [0, SPLIT) -> S_a (AllReduce #1)
SPLIT2 = 56                  # tiles [SPLIT, SPLIT2) -> S_b (AllReduce #2)
PAIR_TAPS = [(0, 0), (0, 1), (1, 0)]     # (ky,kx); partner = (ky+1,kx+1)
RG = [[0, 1], [2, 3], [4, 5], [6, 7]]    # AllReduce pairs (same sample)

_CACHE = {}


def _rhs3(xd_t, parts, j, ky, kx, p0=0):
    """[parts, 2, 256] view: output sub-tile j, tap (ky, kx)."""
    v = xd_t[p0:p0 + parts, :].rearrange("p (r c) -> p r c", r=WINR + 2, c=RS)
    return v[:, 2 * j + ky: 2 * j + ky + 2, kx: kx + 256]


def build_nc():
    nc = bacc.Bacc("TRN2", target_bir_lowering=False, debug=False, num_devices=8)

    xd_d = nc.dram_tensor("xd", [128, XLT], BF16, kind="ExternalInput")
    xd3_d = nc.dram_tensor("xd3", [128, XLT], BF16, kind="ExternalInput")
    wp_d = nc.dram_tensor("wp", [128, 3 * 128], BF16, kind="ExternalInput")
    ws_d = nc.dram_tensor("ws", [128, 2 * 128], BF16, kind="ExternalInput")
    w2_d = nc.dram_tensor("w2", [128, 128], BF16, kind="ExternalInput")
    wo_d = nc.dram_tensor("wo", [64, 64], F32, kind="ExternalInput")
    tp_d = nc.dram_tensor("tp", [1, 1], F32, kind="ExternalInput")
    out_d = nc.dram_tensor("out", [64, N], BF16, kind="ExternalOutput")

    with tile.TileContext(nc) as tc:
        from contextlib import ExitStack
        with ExitStack() as outer:
            pool_w = outer.enter_context(tc.tile_pool(name="wts", bufs=1))
            pool_s = outer.enter_context(tc.tile_pool(name="sbuf_s", bufs=1))
            pool_ps_S = outer.enter_context(
                tc.tile_pool(name="ps_S", bufs=1, space="PSUM"))
            pool_dram = outer.enter_context(
                tc.tile_pool(name="dram", bufs=1, space="DRAM"))

            # persistent tiles
            wp_sb = pool_w.tile([128, 3 * 128], BF16)
            ws_sb = pool_w.tile([128, 2 * 128], BF16)
            w2_sb = pool_w.tile([128, 128], BF16)
            wo_sb = pool_w.tile([64, 64], F32)
            tp_sb = pool_w.tile([1, 1], F32)
            id_bf = pool_w.tile([128, 128], BF16)
            s_t = pool_s.tile([128, N], BF16)
            S_all = pool_ps_S.tile([128, 256], F32)
            S_ps = S_all[:, 0:128]
            S_ps_c = S_all[:, 128:256]
            cc_in_b = pool_dram.tile([66, 64], BF16)
            cc_out_b = pool_dram.tile([66, 64], BF16)
            dmy_in = pool_dram.tile([1, 16], F32)
            dmy_out = pool_dram.tile([1, 16], F32)
            warm_d = pool_dram.tile([1, 16], F32)

            # tiny dummy AllReduce: pays the one-time mesh-algo init on the
            # CC core and absorbs inter-core launch skew while pass 1 runs.
            # Its payload rides HWDGE so the trigger fires within ~5us; a
            # separate throwaway SWDGE DMA pays the ~30us software-DGE
            # cold-start in the background before the real payloads need it.
            dmy_sb = pool_w.tile([1, 16], F32)
            nc.gpsimd.memset(dmy_sb[:], 1.0)
            nc.sync.dma_start(dmy_in[:], dmy_sb[:])
            nc.gpsimd.collective_compute(
                "AllReduce", mybir.AluOpType.add, replica_groups=RG,
                ins=[dmy_in.opt()], outs=[dmy_out.opt()])
            nc.gpsimd.dma_start(warm_d[:], dmy_sb[:])
            # weights ride the scalar queue so the sync queue starts window
            # 0 immediately; wo/tp (needed late) load inside the loop
            nc.scalar.dma_start(wp_sb[:], wp_d[:])
            nc.scalar.dma_start(w2_sb[:], w2_d[:])
            nc.scalar.dma_start(ws_sb[:], ws_d[:])
            make_identity(nc, id_bf[:])
            # preload ACT table sets (exp, abs_rsqrt) so the softmax phase
            # does not pay the ~2.7us-per-set load inside the collective gap
            scr_a = pool_w.tile([1, 1], F32)
            scr_b = pool_w.tile([1, 1], F32)
            nc.scalar.activation(scr_a[:], dmy_sb[0:1, 0:1],
                                 mybir.ActivationFunctionType.Exp)
            nc.scalar.activation(scr_b[:], scr_a[:],
                                 mybir.ActivationFunctionType.Abs_reciprocal_sqrt)
            ones_sb = pool_w.tile([1, 64], F32)
            nc.gpsimd.memset(ones_sb[:], 1.0)
            ones_col = pool_w.tile([128, 1], F32)
            nc.gpsimd.memset(ones_col[:], 1.0)
            tb_sb = pool_w.tile([64, 1], F32)   # temp broadcast column
            # constant f32 diag mask (expanded from bf16 identity)
            diag_msk = pool_w.tile([128, 128], F32)
            nc.scalar.copy(diag_msk[:], id_bf[:])
            # pass-2 weights buffer, zero-padded to K=128
            a2t_bf = pool_w.tile([128, 64], BF16)
            nc.gpsimd.memset(a2t_bf[:], 0.0)

            # ---------------- pass 1: conv front-end + gram ----------------
            # bf16 collective payload: the gram entries only feed softmax
            # logits so the 0.4% rounding is well inside the error budget
            dtmp = pool_w.tile([128, 128], F32)
            diag_b = pool_w.tile([128, 1], BF16)
            diag_c = pool_w.tile([128, 1], F32)
            Sb_sb = pool_w.tile([64, 64], BF16)
            # warm the PE HAM before pass 1: a dense burst of dummy
            # matmuls with (almost) no dependencies that runs during the
            # initial DMA waits
            # warm with the on-chip identity: no DMA dependency, so the
            # burst starts as soon as the sequencers come up
            with tc.tile_pool(name="ps_w0", bufs=1, space="PSUM") as pw0:
                warm0 = pw0.tile([128, 128], F32)
                for _ in range(12):
                    nc.tensor.matmul(warm0[:], id_bf[:], id_bf[:],
                                     start=True, stop=True)

            with ExitStack() as p1:
                pool_xd = p1.enter_context(tc.tile_pool(name="xd", bufs=8))
                pool_tb = p1.enter_context(
                    tc.tile_pool(name="ps_tb", bufs=1, space="PSUM"))
                pool_dw = p1.enter_context(
                    tc.tile_pool(name="ps_dw", bufs=3, space="PSUM"))
                pool_x2 = p1.enter_context(
                    tc.tile_pool(name="ps_x2", bufs=1, space="PSUM"))
                pool_tr = p1.enter_context(
                    tc.tile_pool(name="ps_tr", bufs=2, space="PSUM"))
                pool_x2sb = p1.enter_context(tc.tile_pool(name="x2sb", bufs=4))
                pool_st = p1.enter_context(tc.tile_pool(name="stsb", bufs=6))
                pool_sc = p1.enter_context(tc.tile_pool(name="scsb", bufs=3))

                sT_tiles = {}
                sc_tiles = {}

                def src_of(t):
                    if t < NTILES:
                        return s_t[:, NT * t: NT * (t + 1)]
                    return sc_tiles[t][:]

                def emit_transpose(t):
                    src = src_of(t)
                    tr_ps = pool_tr.tile([128, NT], BF16)
                    for q in range(4):
                        nc.tensor.transpose(
                            tr_ps[:, 128 * q: 128 * (q + 1)],
                            src[:, 128 * q: 128 * (q + 1)],
                            id_bf[:])
                    sT_sb = pool_st.tile([128, NT], BF16)
                    nc.vector.tensor_copy(sT_sb[:], tr_ps[:])
                    sT_tiles[t] = sT_sb
                    if t >= NTILES:
                        sc_tiles.pop(t)

                def emit_gram(t):
                    sT_sb = sT_tiles.pop(t)
                    Sdst = S_ps if t < SPLIT2 else S_ps_c
                    for q in range(4):
                        a = sT_sb[:, 128 * q: 128 * (q + 1)]
                        nc.tensor.matmul(
                            Sdst[:], a, a,
                            start=(t in (0, SPLIT2) and q == 0),
                            stop=(t in (SPLIT2 - 1, TTILES - 1) and q == 3))
                    if t == SPLIT2 - 1:
                        # evacuate the tiles-0..55 gram (Gvq block + diag,
                        # pre-scaled by temperature) and start the single
                        # AllReduce; it completes under the tail compute.
                        # Payload DMAs ride the SWDGE (gpsimd) queue so
                        # they never block window loads on HWDGE queues.
                        nc.vector.tensor_scalar(
                            out=Sb_sb[:], in0=S_ps[0:64, 64:128],
                            scalar1=tb_sb[:], scalar2=None,
                            op0=mybir.AluOpType.mult)
                        nc.vector.tensor_mul(dtmp[:], S_ps[:], diag_msk[:])
                        with nc.allow_low_precision(
                                reason="bf16 collective payload"):
                            nc.vector.tensor_reduce(
                                diag_b[:], dtmp[:],
                                axis=mybir.AxisListType.X,
                                op=mybir.AluOpType.add)
                        nc.gpsimd.dma_start(cc_in_b[0:64, 0:64], Sb_sb[:])
                        nc.gpsimd.dma_start(cc_in_b[64:66, 0:64], diag_b[:])
                        nc.gpsimd.collective_compute(
                            "AllReduce", mybir.AluOpType.add,
                            replica_groups=RG,
                            ins=[cc_in_b.opt()], outs=[cc_out_b.opt()])

                for w in range(NWIN + NWIN_T):
                    xd_t = pool_xd.tile([128, WIN], BF16)
                    xd3_t = pool_xd.tile([128, WIN], BF16)
                    if w < NWIN:
                        base = w * WINR * RS
                    else:
                        base = XLEN + (w - NWIN) * WINR * RS
                    if w <= 1:
                        # fine split so the first sub-tiles' taps land fast
                        nc.sync.dma_start(
                            xd_t[:, 0:1032], xd_d[:, base: base + 1032])
                        nc.scalar.dma_start(
                            xd3_t[:, 0:1032], xd3_d[:, base: base + 1032])
                        nc.sync.dma_start(
                            xd_t[:, 1032:WIN], xd_d[:, base + 1032: base + WIN])
                        nc.scalar.dma_start(
                            xd3_t[:, 1032:WIN],
                            xd3_d[:, base + 1032: base + WIN])
                    else:
                        nc.sync.dma_start(
                            xd_t[:], xd_d[:, base: base + WIN])
                        nc.scalar.dma_start(
                            xd3_t[:], xd3_d[:, base: base + WIN])
                    if w == 1:
                        # late weights, off the startup critical path
                        nc.sync.dma_start(wo_sb[:], wo_d[:])
                        nc.sync.dma_start(tp_sb[:], tp_d[:])
                    if w == 3:
                        # temperature broadcast column (needed at SPLIT-1)
                        tb_ps = pool_tb.tile([64, 1], F32)
                        nc.tensor.matmul(tb_ps[:], ones_sb[:], tp_sb[:],
                                         start=True, stop=True)
                        nc.vector.tensor_copy(tb_sb[:], tb_ps[:])
                    for j in range(SUBT):
                        t = SUBT * w + j
                        # x2 first: its ACT evacuation overlaps the conv MMs
                        x2_ps = pool_x2.tile([128, NT], F32)
                        nc.tensor.matmul(
                            x2_ps[:], w2_sb[:],
                            _rhs3(xd_t, 128, j, 1, 1),
                            start=True, stop=True)
                        x2_sb = pool_x2sb.tile([128, NT], F32)
                        nc.scalar.copy(x2_sb[:], x2_ps[:])
                        dw_ps = pool_dw.tile([128, NT], F32)
                        for p, (ky, kx) in enumerate(PAIR_TAPS):
                            rhs = _rhs3(xd_t, 128, j, ky, kx)
                            nc.tensor.matmul(
                                dw_ps[:],
                                wp_sb[:, 128 * p: 128 * (p + 1)],
                                rhs,
                                start=(p == 0), stop=False)
                        # xd3 low half = x+516 (tap (2,0)), high = x+2
                        # (tap (0,2)): one K=128 pair, then (2,2) single
                        nc.tensor.matmul(
                            dw_ps[:], ws_sb[:, 0:128],
                            _rhs3(xd3_t, 128, j, 0, 0),
                            start=False, stop=False)
                        nc.tensor.matmul(
                            dw_ps[:], ws_sb[:, 128:256],
                            _rhs3(xd3_t, 128, j, 0, 2),
                            start=False, stop=True)
                        # PE fills the wait for this tile's DVE mult with
                        # last tile's transposes and an older gram
                        if t >= 1:
                            emit_transpose(t - 1)
                        if t >= 2:
                            emit_gram(t - 2)
                        if t < NTILES:
                            dst = s_t[:, NT * t: NT * (t + 1)]
                        else:
                            sc_sb = pool_sc.tile([128, NT], BF16)
                            sc_tiles[t] = sc_sb
                            dst = sc_sb[:]
                        nc.vector.tensor_mul(dst, dw_ps[:], x2_sb[:])
                emit_transpose(TTILES - 1)
                emit_gram(TTILES - 2)
                emit_gram(TTILES - 1)

            # ---------------- local tail gram (no collective) ----------------
            gvq_c = pool_w.tile([64, 64], F32)
            nc.vector.tensor_scalar(
                out=gvq_c[:], in0=S_ps_c[0:64, 64:128],
                scalar1=tb_sb[:], scalar2=None, op0=mybir.AluOpType.mult)
            nc.vector.tensor_mul(dtmp[:], S_ps_c[:], diag_msk[:])

            # readback of AR#2
            gvq_b = pool_w.tile([64, 64], BF16)
            sv_b = pool_w.tile([1, 64], BF16)
            sq_b = pool_w.tile([1, 64], BF16)
            nc.sync.dma_start(gvq_b[:], cc_out_b[0:64, :])
            nc.scalar.dma_start(sv_b[:], cc_out_b[64:65, :])
            nc.scalar.dma_start(sq_b[:], cc_out_b[65:66, :])

            # ---------------- softmax + fused output weights ----------------
            with ExitStack() as p15:
                ps_sm = p15.enter_context(
                    tc.tile_pool(name="ps_sm", bufs=1, space="PSUM"))
                # local diag directly as a row: ones^T @ (S_c * I)
                dgT_ps = ps_sm.tile([1, 128], F32)
                nc.tensor.matmul(dgT_ps[:], ones_col[:], dtmp[:],
                                 start=True, stop=True)
                gvq_sb = pool_w.tile([64, 64], F32)     # [d, c] = v_d . q_c
                sv_s = pool_w.tile([1, 64], F32)
                sq_s = pool_w.tile([1, 64], F32)
                nc.vector.tensor_add(gvq_sb[:], gvq_b[:], gvq_c[:])
                nc.vector.tensor_add(sv_s[:], sv_b[:], dgT_ps[0:1, 0:64])
                nc.vector.tensor_add(sq_s[:], sq_b[:], dgT_ps[0:1, 64:128])
                # R[c,d] = rsqrt(sq[c] * sv[d]); temp is already in Gvq
                P_ps = ps_sm.tile([64, 64], F32)
                nc.tensor.matmul(P_ps[:], sq_s[:], sv_s[:],
                                 start=True, stop=True)
                Rm = pool_w.tile([64, 64], F32)
                nc.scalar.activation(
                    Rm[:], P_ps[:],
                    mybir.ActivationFunctionType.Abs_reciprocal_sqrt)
                # Gqv = Gvq^T
                gqv_ps = ps_sm.tile([64, 64], F32)
                nc.tensor.transpose(gqv_ps[:], gvq_sb[:],
                                    diag_msk[0:64, 0:64])
                # z = Gqv * R;  |z| <= temp so exp needs no max-shift
                z = pool_w.tile([64, 64], F32)
                nc.vector.tensor_mul(z[:], gqv_ps[:], Rm[:])
                e = pool_w.tile([64, 64], F32)
                sums = pool_w.tile([64, 1], F32)
                nc.scalar.activation(
                    e[:], z[:], mybir.ActivationFunctionType.Exp,
                    accum_out=sums[:])
                rs = pool_w.tile([64, 1], F32)
                nc.vector.reciprocal(rs[:], sums[:])
                attn = pool_w.tile([64, 64], F32)
                nc.vector.tensor_scalar(
                    out=attn[:], in0=e[:], scalar1=rs[:], scalar2=None,
                    op0=mybir.AluOpType.mult)
                # A2T = attn^T @ w_out^T  ->  [d, o], K=128-padded in bf16
                a2t_ps = ps_sm.tile([64, 64], F32)
                nc.tensor.matmul(a2t_ps[:], attn[:], wo_sb[:],
                                 start=True, stop=True)
                nc.vector.tensor_copy(a2t_bf[0:64, :], a2t_ps[:])

            # ---------------- pass 2: out = A2 @ v, streamed ----------------
            with ExitStack() as p2:
                ps_o = p2.enter_context(
                    tc.tile_pool(name="ps_o", bufs=7, space="PSUM"))
                ob_pool = p2.enter_context(tc.tile_pool(name="ob", bufs=6))
                # col-tiled pairs: partition half 0 covers output cols
                # [0, N/2), half 1 covers [N/2, N) -- the store DMAs are
                # then fully contiguous per channel row
                HB = N // 2
                BIG = 2048
                for T in range(HB // BIG):
                    ob_sb = ob_pool.tile([128, BIG], BF16)
                    for j in range(4):
                        k = (BIG // NT) * T + j
                        ps = ps_o.tile([128, NT], F32)
                        nc.tensor.matmul(
                            ps[0:64, :], a2t_bf[:],
                            s_t[:, NT * k: NT * (k + 1)],
                            start=True, stop=True)
                        nc.tensor.matmul(
                            ps[64:128, :], a2t_bf[:],
                            s_t[:, HB + NT * k: HB + NT * (k + 1)],
                            start=True, stop=True, tile_position=(0, 64))
                        if j % 2 == 0:
                            nc.scalar.copy(
                                ob_sb[:, NT * j: NT * (j + 1)], ps[:])
                        else:
                            nc.vector.tensor_copy(
                                ob_sb[:, NT * j: NT * (j + 1)], ps[:])
                    nc.sync.dma_start(
                        out_d[0:64, BIG * T: BIG * (T + 1)], ob_sb[0:64])
                    nc.scalar.dma_start(
                        out_d[0:64, HB + BIG * T: HB + BIG * (T + 1)],
                        ob_sb[64:128])

    nc.compile()
    return nc


def _get_nc():
    if "nc" not in _CACHE:
        _CACHE["nc"] = build_nc()
    return _CACHE["nc"]


def _dup_shift(xp):
    """[64, rows, 258] zero-padded block -> the xd / xd3 dup layouts."""
    xlen = xp.shape[1] * RS
    xpf = xp.reshape(64, xlen)
    xpf_pad = np.pad(xpf, ((0, 0), (0, 520)))
    xd = np.concatenate([xpf, xpf_pad[:, SHIFT:SHIFT + xlen]], axis=0)
    xd3 = np.concatenate(
        [xpf_pad[:, 516:516 + xlen], xpf_pad[:, 2:2 + xlen]], axis=0)
    return xd, xd3


def _prep_in_maps(x, w_in, w_dw, w_out, temperature):
    x = np.ascontiguousarray(x, dtype=np.float32)
    w_in = np.asarray(w_in, dtype=np.float32)
    w_dw = np.asarray(w_dw, dtype=np.float32)
    w_out = np.asarray(w_out, dtype=np.float32)
    temp = np.asarray(temperature, dtype=np.float32).reshape(1, 1)

    perm = np.concatenate([np.arange(64, 128), np.arange(0, 64)])
    W_in1 = w_in[:2 * C]          # [128, 64]
    W_in2 = w_in[2 * C:]          # [128, 64]
    wd = w_dw[:, 0]               # [128, 3, 3]

    import ml_dtypes
    bf = ml_dtypes.bfloat16

    wp = np.empty((128, 3 * 128), dtype=np.float32)
    for p, (ky, kx) in enumerate(PAIR_TAPS):
        wp[:64, 128 * p:128 * (p + 1)] = \
            (W_in1[perm] * wd[perm, ky, kx][:, None]).T
        wp[64:, 128 * p:128 * (p + 1)] = \
            (W_in1[perm] * wd[perm, ky + 1, kx + 1][:, None]).T
    # ws col block 0: K=128 pair [(2,0) @ rows 0-63, (0,2) @ rows 64-127]
    # (xd3 low half reads tap (2,0) at view (0,0), high half tap (0,2));
    # col block 1: single (2,2) @ rows 0-63 (xd3 low half, view (0,2))
    ws = np.zeros((128, 2 * 128), dtype=np.float32)
    ws[0:64, 0:128] = (W_in1[perm] * wd[perm, 2, 0][:, None]).T
    ws[64:128, 0:128] = (W_in1[perm] * wd[perm, 0, 2][:, None]).T
    ws[0:64, 128:256] = (W_in1[perm] * wd[perm, 2, 2][:, None]).T
    w2 = np.zeros((128, 128), dtype=np.float32)
    w2[0:64, :] = W_in2[perm].T
    wo = np.ascontiguousarray(w_out.T)              # [64, 64]
    wp = wp.astype(bf)
    ws = ws.astype(bf)
    w2 = w2.astype(bf)

    in_maps = []
    for core in range(8):
        sample, half = core // 2, core % 2
        h0 = half * 128
        xp = np.zeros((64, HROWS, RS), dtype=bf)
        lo, hi = max(h0 - 1, 0), min(h0 + 129, H)
        xp[:, lo - (h0 - 1): hi - (h0 - 1), 1:257] = x[sample, :, lo:hi, :]
        xd, xd3 = _dup_shift(xp)
        # partner tail: last 16 rows of the partner's half (+halo)
        p0 = (1 - half) * 128 + 128 - NWIN_T * WINR
        xq = np.zeros((64, TROWS, RS), dtype=bf)
        qlo, qhi = p0 - 1, min(p0 + NWIN_T * WINR + 1, H)
        xq[:, 0: qhi - qlo, 1:257] = x[sample, :, qlo:qhi, :]
        xdt, xd3t = _dup_shift(xq)
        in_maps.append({
            "xd": np.ascontiguousarray(
                np.concatenate([xd, xdt], axis=1)),
            "xd3": np.ascontiguousarray(
                np.concatenate([xd3, xd3t], axis=1)),
            "wp": wp, "ws": ws, "w2": w2, "wo": wo, "tp": temp,
        })
    return in_maps


def _assemble(results):
    out = np.empty((B, C, H, W), dtype=np.float32)
    for core in range(8):
        sample, half = core // 2, core % 2
        out[sample, :, half * 128: half * 128 + 128, :] = \
            results[core]["out"].astype(np.float32).reshape(C, 128, W)
    return out


def run(trace=False, trace_cores=None, **inputs):
    if trace:
        _install_ntff_hook()
    nc = _get_nc()
    in_maps = _prep_in_maps(**inputs)
    res = run_bass_kernel_spmd(nc, in_maps, core_ids=list(range(8)),
                               trace=trace, trace_cores=trace_cores)
    if trace and res.mean_exec_time_ns:
        print(f"mean exec {res.mean_exec_time_ns/1000:.1f}us, "
              f"max core {res.max_exec_time_core_id}")
    return _assemble(res.results), res.exec_time_ns


def kernel(**inputs) -> np.ndarray:
    out, _ = run(trace=False, **inputs)
    return out
